# revision 41
# baseline (speedup 1.0000x reference)
"""Multi-head attention kernel for 8 TRN2 NeuronCores.

Problem: B=2, S=2048, D=1024, H=16 heads, head_dim=64, fp32 I/O.

Sharding (per the tensor-parallel hint): 8 cores = 2 batches x 4 head-groups.
Core c handles batch c//4 and heads [4*(c%4), 4*(c%4)+4). Each core:
  - projects its head-slice qT/kT (feature-on-partition layout, 2 heads per
    128-partition tile) and v (natural layout, with an appended ones column),
  - computes scoresT = k @ q.T per head with K=64 row-tiled matmuls (two heads
    run concurrently in the PE array),
  - exp on ScalarE with the 1/sqrt(64) scale and the -1e9 mask folded into the
    activation's scale/bias,
  - attn@v with the [v|1] trick: the ones column makes the softmax denominator
    fall out of the same matmul stream (PSUM row 64),
  - normalizes via a direct bf16 reciprocal on the PSUM Z row plus a rank-1
    PE broadcast matmul, deferred into the next block's early jc slots so the
    PE stream never stalls at ic-block boundaries,
  - computes a partial output projection over its 256 features.
Host sums the 4 partials per batch and adds the output bias.
All matmul operands are bf16 (fp32 matmul is 4x slower on the PE array);
accumulation is fp32 in PSUM and the returned partials are fp32.

Input streaming: x arrives host-packed s-chunk-major so each 512-wide s-chunk
of all 8 d-chunks is ONE contiguous 1MB DMA (8KB per partition). All input
DMAs ride the Sync HWDGE ring in need-order; a single queue saturates HBM
(an InstDMACopy spreads across all 16 SDMA engines) and keeps the Scalar
(ACT) queue free for the exp stream. ~90 tiny junk matmuls at t=0 keep the
PE HAM activity monitor busy during the initial DMA wait so the prefix
projections run at the warm 2.4GHz clock instead of cold 1.2GHz.
"""

import numpy as np
import ml_dtypes

import concourse.mybir as mybir
import concourse.tile as tile
from concourse import bacc
from concourse.bass_utils import run_bass_kernel_spmd

BF16 = mybir.dt.bfloat16
FP32 = mybir.dt.float32

B, S, D = 2, 2048, 1024
NH, DH = 16, 64
NCORES = 8
GROUPS = 4                 # head-groups (cores per batch)
HL = NH // GROUPS          # heads per core = 4
FL = HL * DH               # features per core = 256
NPAIR = HL // 2            # head pairs per core = 2

SC = 512                   # i/s chunk (PSUM bank = 512 fp32)
JC = 128                   # j chunk (partition dim)
DCH = D // 128             # contraction chunks over embed dim = 8
N_SC = S // SC             # 4
N_JC = S // JC             # 16

# bias_all fp32 packing: bk | bq | bv | mb
BK_OFF, BQ_OFF, BV_OFF, MB_OFF = 0, 2, 4, 4 + FL
BIAS_COLS = 4 + FL + N_JC


def build_kernel():
    nc = bacc.Bacc("TRN2", target_bir_lowering=False, debug=False)

    # x host-packed s-chunk-major: xP[p, sc*4096 + dc*512 + j] = x[sc*512+j, dc*128+p]
    xP = nc.dram_tensor("xP", [128, N_SC * DCH * SC], BF16, kind="ExternalInput")
    wk = nc.dram_tensor("wk", [128, DCH * FL], BF16, kind="ExternalInput")
    wq = nc.dram_tensor("wq", [128, DCH * FL], BF16, kind="ExternalInput")
    wv = nc.dram_tensor("wv", [128, DCH * FL], BF16, kind="ExternalInput")
    wo = nc.dram_tensor("wo", [128, 2 * D], BF16, kind="ExternalInput")
    bias = nc.dram_tensor("bias", [128, BIAS_COLS], FP32, kind="ExternalInput")
    out = nc.dram_tensor("out", [S, D], FP32, kind="ExternalOutput")

    with tile.TileContext(nc) as tc:
        with (
            tc.tile_pool(name="weights", bufs=1) as wpool,
            tc.tile_pool(name="acts", bufs=1) as apool,
            tc.tile_pool(name="exps", bufs=20) as epool,
            tc.tile_pool(name="stages", bufs=6) as spool,
            tc.tile_pool(name="smalls", bufs=6) as smpool,
            tc.tile_pool(name="scores", bufs=2, space="PSUM") as scpool,
            tc.tile_pool(name="attnout", bufs=2, space="PSUM") as aopool,
            tc.tile_pool(name="projacc", bufs=2, space="PSUM") as prpool,
        ):
            # ---- PE warm-up: full-array junk matmuls keep HAM's activity
            # window busy while the first DMAs stream, so the real prefix
            # runs at 2.4GHz. (K=1 slivers don't register as activity; the
            # N=512 streams give ~100% duty cycle. 8 of them = ~3.4us = one
            # full HAM activity window, ending right as the first x chunk
            # lands.)
            wtiny = wpool.tile([128, SC], BF16, name="wtiny")
            nc.vector.memset(wtiny, 0.0)

            def warm_pe(n):
                warm_ps = prpool.tile([128, SC], FP32, name="warm_ps", tag="ps")
                for _ in range(n):
                    nc.tensor.matmul(warm_ps, lhsT=wtiny[:, 0:128], rhs=wtiny)

            warm_pe(14)

            # ---- resident inputs: all on the Sync HWDGE ring, need-ordered.
            xt_all = wpool.tile([128, N_SC * DCH * SC], BF16, name="xt_all")

            def xt(dc, sc):
                base = sc * (DCH * SC) + dc * SC
                return xt_all[:, base:base + SC]

            wk_sb = wpool.tile([128, DCH * FL], BF16, name="wk_sb")
            wkt = [wk_sb[:, dc * FL:(dc + 1) * FL] for dc in range(DCH)]
            wq_sb = wpool.tile([128, DCH * FL], BF16, name="wq_sb")
            wqt = [wq_sb[:, dc * FL:(dc + 1) * FL] for dc in range(DCH)]
            bias_sb = wpool.tile([128, BIAS_COLS], FP32, name="bias_sb")
            bk_sb = bias_sb[:, BK_OFF:BK_OFF + 2]
            bq_sb = bias_sb[:, BQ_OFF:BQ_OFF + 2]
            bv_sb = bias_sb[:, BV_OFF:BV_OFF + FL]
            mb_sb = bias_sb[:, MB_OFF:MB_OFF + N_JC]
            wv_sb = wpool.tile([128, DCH * FL], BF16, name="wv_sb")
            wvt = [wv_sb[:, dc * FL:(dc + 1) * FL] for dc in range(DCH)]
            wo_sb = wpool.tile([128, 2 * D], BF16, name="wo_sb")
            wot = [wo_sb[:, fc * D:(fc + 1) * D] for fc in range(2)]

            half = DCH * SC // 2
            # q before k: the first scores need ALL of q s-chunk 0 but only
            # the first columns of kT, which a column-split k projection
            # produces right as wk (ordered last of the four) lands
            nc.sync.dma_start(out=wq_sb, in_=wq.ap())
            # s-chunk 0 in dc halves so the q projection halves can start as
            # soon as their contraction operands land
            nc.sync.dma_start(out=xt_all[:, 0:half], in_=xP.ap()[:, 0:half])
            nc.sync.dma_start(out=xt_all[:, half:2 * half],
                              in_=xP.ap()[:, half:2 * half])
            nc.sync.dma_start(out=wk_sb, in_=wk.ap())
            nc.sync.dma_start(out=bias_sb, in_=bias.ap())
            nc.sync.dma_start(out=wv_sb, in_=wv.ap())
            for sc in range(1, N_SC):
                nc.sync.dma_start(
                    out=xt_all[:, sc * DCH * SC:(sc + 1) * DCH * SC],
                    in_=xP.ap()[:, sc * DCH * SC:(sc + 1) * DCH * SC],
                )
            nc.sync.dma_start(out=wo_sb, in_=wo.ap())
            # lower half of wot[1] shifted to partitions 0-63: lets the tail
            # out_proj contract head B's staging tile directly (K=64) instead
            # of waiting for the partition-shift DMA into at[1]
            wo_lo = wpool.tile([64, D], BF16, name="wo_lo")
            nc.sync.dma_start(out=wo_lo, in_=wo_sb[64:128, D:2 * D])

            # warm the ScalarE Exp table while DMAs stream
            warm = smpool.tile([1, 4], FP32, name="warm", tag="warm")
            nc.vector.memset(warm, 1.0)
            nc.scalar.activation(warm, warm, mybir.ActivationFunctionType.Exp)

            # ---- persistent activations ----
            qt = [apool.tile([128, S], BF16, name=f"qt{p}") for p in range(2)]
            kt = [apool.tile([128, S], BF16, name=f"kt{p}") for p in range(2)]
            # v natural: tile sc = rows [128sc,128sc+128), layout (128, 4 heads, 65)
            vt = [apool.tile([128, HL, 65], BF16, name=f"vt{sc}") for sc in range(N_JC)]
            at = [apool.tile([128, S], BF16, name=f"at{p}") for p in range(2)]

            qk_open = {}

            def qk_half(dst, w_tiles, bias_ap, sc, fc, half):
                """Half of a qT/kT projection s-chunk (4 of 8 d-accumulation
                matmuls, ~0.9us of PE) so drip slots stay small."""
                key = (id(dst), sc)
                if half == 0:
                    ps = prpool.tile([128, SC], FP32, name="ps", tag="ps")
                    qk_open[key] = ps
                else:
                    ps = qk_open.pop(key)
                for dc in range(half * 4, half * 4 + 4):
                    nc.tensor.matmul(
                        ps,
                        lhsT=w_tiles[dc][:, fc * 128:(fc + 1) * 128],
                        rhs=xt(dc, sc),
                        start=(dc == 0),
                        stop=(dc == DCH - 1),
                    )
                if half == 1:
                    nc.vector.tensor_scalar_add(
                        dst[:, sc * SC:(sc + 1) * SC], ps, bias_ap[:, fc:fc + 1]
                    )

            def qk_full(dst, w_tiles, bias_ap, sc, fc):
                qk_half(dst, w_tiles, bias_ap, sc, fc, 0)
                qk_half(dst, w_tiles, bias_ap, sc, fc, 1)

            def qk_cols(dst, w_tiles, bias_ap, sc, fc, c0, c1):
                """Column slice of a qT/kT projection s-chunk (all 8 dc)."""
                ps = prpool.tile([128, c1 - c0], FP32, name="ps", tag="ps")
                for dc in range(DCH):
                    nc.tensor.matmul(
                        ps,
                        lhsT=w_tiles[dc][:, fc * 128:(fc + 1) * 128],
                        rhs=xt(dc, sc)[:, c0:c1],
                        start=(dc == 0),
                        stop=(dc == DCH - 1),
                    )
                nc.vector.tensor_scalar_add(
                    dst[:, sc * SC + c0:sc * SC + c1], ps, bias_ap[:, fc:fc + 1]
                )

            def v_proj(j):
                """v rows [128j,+128) for all 4 heads (N=256, ~1.05us)."""
                sc, off = j // 4, (j % 4) * JC
                ps = prpool.tile([128, FL], FP32, name="ps", tag="ps")
                for dc in range(DCH):
                    nc.tensor.matmul(
                        ps,
                        lhsT=xt(dc, sc)[:, off:off + JC],
                        rhs=wvt[dc],
                        start=(dc == 0),
                        stop=(dc == DCH - 1),
                    )
                nc.vector.tensor_add(
                    vt[j][:, :, 0:64],
                    ps.rearrange("p (h d) -> p h d", h=HL),
                    bv_sb.rearrange("p (h d) -> p h d", h=HL),
                )
                nc.vector.memset(vt[j][:, :, 64:65], 1.0)

            pending_norm = {}
            last_stg = [None]

            def attention(pair, per_jc_hook=None, final=False):
                """Full attention for heads (2*pair, 2*pair+1)."""
                for ic in range(N_SC):
                    i_sl = slice(ic * SC, (ic + 1) * SC)
                    outA = aopool.tile([65, SC], FP32, name="outA", tag="ao")
                    outB = aopool.tile([65, SC], FP32, name="outB", tag="ao")
                    for jc in range(N_JC):
                        sc_ps = scpool.tile([128, 2 * SC], FP32, name="sc_ps")
                        # scoresT = k @ q.T, two heads row-tiled (K=64 each)
                        nc.tensor.matmul(
                            sc_ps[:, 0:SC],
                            lhsT=kt[pair][0:64, jc * JC:(jc + 1) * JC],
                            rhs=qt[pair][0:64, i_sl],
                        )
                        nc.tensor.matmul(
                            sc_ps[:, SC:2 * SC],
                            lhsT=kt[pair][64:128, jc * JC:(jc + 1) * JC],
                            rhs=qt[pair][64:128, i_sl],
                        )
                        ex = epool.tile([128, 2 * SC], BF16, name="ex")
                        nc.scalar.activation(
                            ex, sc_ps, mybir.ActivationFunctionType.Exp,
                            bias=mb_sb[:, jc:jc + 1], scale=1.0 / np.sqrt(DH),
                        )
                        # deferred normalize of the PREVIOUS block: the
                        # copy + reciprocal DMA round trip finishes ~5us into
                        # this block, so the GpSimd partition-broadcast +
                        # normalize muls land at jc slots 5-6 with nothing on
                        # the PE stream at all. MUST be emitted before the
                        # hook work (out_proj drips read `at`).
                        for key, slot in (("b", 5), ("c", 6)):
                            if jc == slot and key in pending_norm:
                                pending_norm.pop(key)()
                        if per_jc_hook is not None:
                            per_jc_hook(ic, jc)
                        nc.tensor.matmul(
                            outA, lhsT=vt[jc][:, 2 * pair, :], rhs=ex[:, 0:SC],
                            start=(jc == 0), stop=(jc == N_JC - 1),
                        )
                        nc.tensor.matmul(
                            outB, lhsT=vt[jc][:, 2 * pair + 1, :], rhs=ex[:, SC:2 * SC],
                            start=(jc == 0), stop=(jc == N_JC - 1),
                        )
                    # Boundary work kept minimal: bounce the PSUM Z rows to a
                    # (64,8) shape via DMA so the reciprocal runs at 8 elems
                    # per lane (233ns; a 1-partition reciprocal costs 3.3us),
                    # then DMA-cast back to a bf16 row. The ~4us round trip
                    # is hidden: the broadcast matmul + normalize mul are
                    # deferred into the next block's jc slots 3-4. PSUM->SBUF
                    # copies recycle the attn-out banks.
                    osbA = smpool.tile([65, SC], FP32, name="osbA", tag="osb")
                    nc.vector.tensor_copy(osbA, outA)
                    osbB = smpool.tile([65, SC], FP32, name="osbB", tag="osb")
                    nc.vector.tensor_copy(osbB, outB)
                    recA = smpool.tile([1, SC], BF16, name="recA", tag="recbf")
                    recB = smpool.tile([1, SC], BF16, name="recB", tag="recbf")
                    # bounce DMAs ride the Sync HWDGE ring (~0.6us first-byte
                    # vs ~2us on the GpSimd SWDGE path). Both forward bounces
                    # issue before either return bounce: the return waits on
                    # its reciprocal, and the Sync ring is FIFO — issuing
                    # zspB behind recA-back would head-of-line block it.
                    zspA = smpool.tile([64, SC // 64], FP32, name="zspA", tag="zsp")
                    zspB = smpool.tile([64, SC // 64], FP32, name="zspB", tag="zsp")
                    nc.sync.dma_start(out=zspA, in_=osbA[64:65, :])
                    nc.sync.dma_start(out=zspB, in_=osbB[64:65, :])
                    rspA = smpool.tile([64, SC // 64], BF16, name="rspA", tag="rsp")
                    rspB = smpool.tile([64, SC // 64], BF16, name="rspB", tag="rsp")
                    with nc.allow_low_precision(reason="1/Z is bf16 by design"):
                        nc.vector.reciprocal(rspA, zspA)
                        nc.vector.reciprocal(rspB, zspB)
                    nc.sync.dma_start(out=recA, in_=rspA)
                    nc.sync.dma_start(out=recB, in_=rspB)

                    is_final = final and ic == N_SC - 1

                    def t2b(pair=pair, i_sl=i_sl, osbA=osbA, recA=recA):
                        bcs = smpool.tile([64, SC], BF16, name="bcs", tag="bcs")
                        nc.gpsimd.partition_broadcast(bcs, recA)
                        nc.vector.tensor_mul(at[pair][0:64, i_sl], osbA[0:64, :], bcs)

                    def t2c(pair=pair, i_sl=i_sl, osbB=osbB, recB=recB,
                            is_final=is_final):
                        bcs = smpool.tile([64, SC], BF16, name="bcs", tag="bcs")
                        nc.gpsimd.partition_broadcast(bcs, recB)
                        stg = smpool.tile([64, SC], BF16, name="stg", tag="stg")
                        nc.vector.tensor_mul(stg, osbB[0:64, :], bcs)
                        if is_final:
                            # tail out_proj reads stg directly (3-matmul form)
                            last_stg[0] = stg
                        else:
                            # shift to partitions 64..127 (DVE can't cross lanes)
                            nc.sync.dma_start(out=at[pair][64:128, i_sl], in_=stg)

                    pending_norm["b"] = t2b
                    pending_norm["c"] = t2c

            def flush_norm():
                for key in ("b", "c"):
                    if key in pending_norm:
                        pending_norm.pop(key)()

            def out_proj_chunk(ic, ec, ss, dma_eng=None, copy_eng=None):
                """One (128 s, 512 e) chunk of the partial output projection."""
                srow = ic * SC + ss * 128
                po = prpool.tile([128, SC], FP32, name="po", tag="ps")
                for fc in range(2):
                    nc.tensor.matmul(
                        po,
                        lhsT=at[fc][:, srow:srow + 128],
                        rhs=wot[fc][:, ec * SC:(ec + 1) * SC],
                        start=(fc == 0),
                        stop=(fc == 1),
                    )
                stg = spool.tile([128, SC], FP32, name="ostg")
                if copy_eng is nc.scalar:
                    nc.scalar.copy(stg, po)
                else:
                    nc.vector.tensor_copy(stg, po)
                (dma_eng or nc.sync).dma_start(
                    out=out.ap()[srow:srow + 128, ec * SC:(ec + 1) * SC],
                    in_=stg,
                )

            # ---- emission order (drives scheduling priority and the
            # per-engine instruction streams; engines execute in order) ----
            #
            # 8 attention blocks (pair, ic). Projection / out-proj work beyond
            # a minimal prefix drips into the jc loops:
            #   block 0: all v chunks just-in-time (vt[j] by jc=j) + k0
            #            s-chunks just-in-time (sc s by jc=4s) + q0 sc1
            #   blocks 1-4: q0/k1/q1 halves (light; blocks run at exp pace)
            #   blocks 5-7: previous ic's out_proj chunks + q1 leftovers
            K0, Q0, K1, Q1 = (kt[0], wkt, bk_sb, 0), (qt[0], wqt, bq_sb, 0), \
                             (kt[1], wkt, bk_sb, 1), (qt[1], wqt, bq_sb, 1)

            def qk_thunk(args, scn, half):
                dst, w, b, fc = args
                return lambda: qk_half(dst, w, b, scn, fc, half)

            sched = {b: {} for b in range(8)}

            def put(b, jc, thunk):
                sched[b].setdefault(jc, []).append(thunk)

            # block 0: v streaming just-in-time at slot j (attn@v jc waits on
            # ACT(jc) which trails scores(jc) by ~1.1us, so v_proj(j) emitted
            # right after scores(j) still lands in time and never delays the
            # next scores) + k0 halves ahead of their first consumer (kt sc s
            # is read from jc=4s).
            for j in range(0, N_JC):
                put(0, j, lambda j=j: v_proj(j))
            put(0, 2, qk_thunk(K0, 1, 0)); put(0, 3, qk_thunk(K0, 1, 1))
            put(0, 6, qk_thunk(K0, 2, 0)); put(0, 7, qk_thunk(K0, 2, 1))
            put(0, 10, qk_thunk(K0, 3, 0)); put(0, 11, qk_thunk(K0, 3, 1))
            put(0, 13, qk_thunk(Q0, 1, 0)); put(0, 14, qk_thunk(Q0, 1, 1))
            # blocks 1-4: spread remaining projections (all light)
            put(1, 4, qk_thunk(Q0, 2, 0)); put(1, 5, qk_thunk(Q0, 2, 1))
            put(1, 8, qk_thunk(Q0, 3, 0)); put(1, 9, qk_thunk(Q0, 3, 1))
            put(1, 12, qk_thunk(K1, 0, 0)); put(1, 13, qk_thunk(K1, 0, 1))
            put(2, 4, qk_thunk(K1, 1, 0)); put(2, 5, qk_thunk(K1, 1, 1))
            put(2, 8, qk_thunk(K1, 2, 0)); put(2, 9, qk_thunk(K1, 2, 1))
            put(2, 12, qk_thunk(K1, 3, 0)); put(2, 13, qk_thunk(K1, 3, 1))
            put(3, 4, qk_thunk(Q1, 0, 0)); put(3, 5, qk_thunk(Q1, 0, 1))
            put(3, 8, qk_thunk(Q1, 1, 0)); put(3, 9, qk_thunk(Q1, 1, 1))
            put(4, 4, qk_thunk(Q1, 2, 0)); put(4, 5, qk_thunk(Q1, 2, 1))
            put(4, 8, qk_thunk(Q1, 3, 0)); put(4, 9, qk_thunk(Q1, 3, 1))
            # blocks 5-7: drip previous ic's out_proj (8 chunks each; slot 7+
            # so the deferred normalize muls at slots 5-6 land first)
            for b in range(5, 8):
                ic_prev = b - 5
                idx = 0
                for ec in range(2):
                    for ss in range(SC // 128):
                        put(b, 7 + idx, lambda ic=ic_prev, ec=ec, ss=ss:
                            out_proj_chunk(ic, ec, ss))
                        idx += 1

            def hook(block):
                def _h(ic, jc):
                    for thunk in sched[block].get(jc, []):
                        thunk()
                return _h

            # minimal prefix: q0 halves paced by the split sc0 DMA arrival,
            # then k0 in column pieces (the first scores only need kT cols
            # 0:128, so a 256-col piece unblocks them right as wk lands).
            # Junk warm matmuls fill the DMA-wait gaps so the HAM activity
            # window never sees the PE idle and everything runs at 2.4GHz.
            qk_half(qt[0], wqt, bq_sb, 0, 0, 0)
            warm_pe(4)
            qk_half(qt[0], wqt, bq_sb, 0, 0, 1)
            warm_pe(4)
            qk_cols(kt[0], wkt, bk_sb, 0, 0, 0, 256)
            qk_cols(kt[0], wkt, bk_sb, 0, 0, 256, 512)

            attention(0, per_jc_hook=lambda ic, jc: hook(ic)(ic, jc))
            attention(1, per_jc_hook=lambda ic, jc: hook(4 + ic)(ic, jc),
                      final=True)
            # tail: keep the PE's HAM activity monitor hot through the ~4us
            # reciprocal round trip (else the final out_proj runs at the cold
            # 1.2GHz clock), then the final block's normalize + out_proj.
            # The final out_proj uses the 3-matmul form (head B contracted
            # from the staging tile via wo_lo) so it starts right after the
            # normalize muls instead of waiting for the partition-shift DMA.
            # Copies and DMAs alternate between engines so neither the
            # Vector queue nor one DMA ring serializes the drain.
            warm_pe(34)
            flush_norm()
            idx = 0
            for ec in range(2):
                for ss in range(SC // 128):
                    srow = (N_SC - 1) * SC + ss * 128
                    po = prpool.tile([128, SC], FP32, name="po", tag="ps")
                    e_sl = slice(ec * SC, (ec + 1) * SC)
                    nc.tensor.matmul(po, lhsT=at[0][:, srow:srow + 128],
                                     rhs=wot[0][:, e_sl], start=True, stop=False)
                    nc.tensor.matmul(po, lhsT=at[1][0:64, srow:srow + 128],
                                     rhs=wot[1][0:64, e_sl], start=False, stop=False)
                    nc.tensor.matmul(
                        po, lhsT=last_stg[0][:, ss * 128:(ss + 1) * 128],
                        rhs=wo_lo[:, e_sl], start=False, stop=True)
                    stg = spool.tile([128, SC], FP32, name="ostg")
                    if idx % 2:
                        nc.scalar.copy(stg, po)
                    else:
                        nc.vector.tensor_copy(stg, po)
                    (nc.scalar if idx % 2 else nc.sync).dma_start(
                        out=out.ap()[srow:srow + 128, e_sl], in_=stg)
                    idx += 1

    nc.compile()
    return nc


_NC_CACHE = None


def _get_nc():
    global _NC_CACHE
    if _NC_CACHE is None:
        _NC_CACHE = build_kernel()
    return _NC_CACHE


def make_in_maps(inputs):
    x = np.asarray(inputs["x"], dtype=np.float32)
    mask = np.asarray(inputs["mask"])
    Wq = np.asarray(inputs["Wq"], dtype=np.float32)
    bq = np.asarray(inputs["bq"], dtype=np.float32)
    Wk = np.asarray(inputs["Wk"], dtype=np.float32)
    bk = np.asarray(inputs["bk"], dtype=np.float32)
    Wv = np.asarray(inputs["Wv"], dtype=np.float32)
    bv = np.asarray(inputs["bv"], dtype=np.float32)
    Wo = np.asarray(inputs["Wo"], dtype=np.float32)

    bf = ml_dtypes.bfloat16

    def pack_dxf(wT):  # (1024, FL) -> (128, 8*FL): d-chunks side by side
        return np.ascontiguousarray(
            wT.reshape(DCH, 128, FL).transpose(1, 0, 2).reshape(128, DCH * FL)
        )

    def pack_fxe(woT):  # (256, D) -> (128, 2*D): f-chunks side by side
        return np.ascontiguousarray(
            woT.reshape(2, 128, D).transpose(1, 0, 2).reshape(128, 2 * D)
        )

    in_maps = []
    for c in range(NCORES):
        b = c // GROUPS
        g = c % GROUPS
        fs, fe = g * FL, (g + 1) * FL
        # x s-chunk-major: xP[p, sc*4096 + dc*512 + j] = x[b][sc*512+j, dc*128+p]
        xPc = np.ascontiguousarray(
            x[b].reshape(N_SC, SC, DCH, 128).transpose(3, 0, 2, 1)
            .reshape(128, N_SC * DCH * SC)
        ).astype(bf)
        bias_c = np.zeros((128, BIAS_COLS), dtype=np.float32)
        bias_c[:, BK_OFF:BK_OFF + 2] = bk[fs:fe].reshape(2, 128).T
        bias_c[:, BQ_OFF:BQ_OFF + 2] = bq[fs:fe].reshape(2, 128).T
        bias_c[:, BV_OFF:BV_OFF + FL] = np.tile(bv[fs:fe], (128, 1))
        bias_c[:, MB_OFF:MB_OFF + N_JC] = (
            np.where(mask[b] == 0, np.float32(-1e9), np.float32(0.0))
            .astype(np.float32).reshape(N_JC, 128).T
        )
        in_maps.append({
            "xP": xPc,
            "wk": pack_dxf(Wk[fs:fe, :].T.astype(bf)),
            "wq": pack_dxf(Wq[fs:fe, :].T.astype(bf)),
            "wv": pack_dxf(Wv[fs:fe, :].T.astype(bf)),
            "wo": pack_fxe(Wo[:, fs:fe].T.astype(bf)),
            "bias": np.ascontiguousarray(bias_c),
        })
    return in_maps


def kernel(x, mask, Wq, bq, Wk, bk, Wv, bv, Wo, bo):
    bo = np.asarray(bo, dtype=np.float32)
    nc = _get_nc()
    in_maps = make_in_maps(dict(x=x, mask=mask, Wq=Wq, bq=bq, Wk=Wk, bk=bk,
                                Wv=Wv, bv=bv, Wo=Wo, bo=bo))
    res = run_bass_kernel_spmd(nc, in_maps, core_ids=list(range(NCORES)))
    parts = [np.asarray(r["out"], dtype=np.float32) for r in res.results]
    full = np.empty((B, S, D), dtype=np.float32)
    for b in range(B):
        acc = parts[b * GROUPS].copy()
        for g in range(1, GROUPS):
            acc += parts[b * GROUPS + g]
        full[b] = acc + bo[None, :]
    return full


# revision 44
# speedup vs baseline: 1.0273x; 1.0273x over previous
"""Multi-head attention kernel for 8 TRN2 NeuronCores.

Problem: B=2, S=2048, D=1024, H=16 heads, head_dim=64, fp32 I/O.

Sharding (per the tensor-parallel hint): 8 cores = 2 batches x 4 head-groups.
Core c handles batch c//4 and heads [4*(c%4), 4*(c%4)+4). Each core:
  - projects its head-slice qT/kT (feature-on-partition layout, 2 heads per
    128-partition tile) and v (natural layout, with an appended ones column),
  - computes scoresT = k @ q.T per head with K=64 row-tiled matmuls (two heads
    run concurrently in the PE array),
  - exp on ScalarE with the 1/sqrt(64) scale and the -1e9 mask folded into the
    activation's scale/bias,
  - attn@v with the [v|1] trick: the ones column makes the softmax denominator
    fall out of the same matmul stream (PSUM row 64),
  - normalizes via a direct bf16 reciprocal on the PSUM Z row plus a rank-1
    PE broadcast matmul, deferred into the next block's early jc slots so the
    PE stream never stalls at ic-block boundaries,
  - computes a partial output projection over its 256 features.
Host sums the 4 partials per batch and adds the output bias.
All matmul operands are bf16 (fp32 matmul is 4x slower on the PE array);
accumulation is fp32 in PSUM and the returned partials are fp32.

Input streaming: x arrives host-packed s-chunk-major so each 512-wide s-chunk
of all 8 d-chunks is ONE contiguous 1MB DMA (8KB per partition). All input
DMAs ride the Sync HWDGE ring in need-order; a single queue saturates HBM
(an InstDMACopy spreads across all 16 SDMA engines) and keeps the Scalar
(ACT) queue free for the exp stream. ~90 tiny junk matmuls at t=0 keep the
PE HAM activity monitor busy during the initial DMA wait so the prefix
projections run at the warm 2.4GHz clock instead of cold 1.2GHz.
"""

import numpy as np
import ml_dtypes

import concourse.mybir as mybir
import concourse.tile as tile
from concourse import bacc
from concourse.bass_utils import run_bass_kernel_spmd

BF16 = mybir.dt.bfloat16
FP32 = mybir.dt.float32

B, S, D = 2, 2048, 1024
NH, DH = 16, 64
NCORES = 8
GROUPS = 4                 # head-groups (cores per batch)
HL = NH // GROUPS          # heads per core = 4
FL = HL * DH               # features per core = 256
NPAIR = HL // 2            # head pairs per core = 2

SC = 512                   # i/s chunk (PSUM bank = 512 fp32)
JC = 128                   # j chunk (partition dim)
DCH = D // 128             # contraction chunks over embed dim = 8
N_SC = S // SC             # 4
N_JC = S // JC             # 16

# bias_all fp32 packing: bk | bq | bv | mb
BK_OFF, BQ_OFF, BV_OFF, MB_OFF = 0, 2, 4, 4 + FL
BIAS_COLS = 4 + FL + N_JC


def build_kernel():
    nc = bacc.Bacc("TRN2", target_bir_lowering=False, debug=False)

    # x host-packed s-chunk-major: xP[p, sc*4096 + dc*512 + j] = x[sc*512+j, dc*128+p]
    xP = nc.dram_tensor("xP", [128, N_SC * DCH * SC], BF16, kind="ExternalInput")
    wk = nc.dram_tensor("wk", [128, DCH * FL], BF16, kind="ExternalInput")
    wq = nc.dram_tensor("wq", [128, DCH * FL], BF16, kind="ExternalInput")
    wv = nc.dram_tensor("wv", [128, DCH * FL], BF16, kind="ExternalInput")
    wo = nc.dram_tensor("wo", [128, 2 * D], BF16, kind="ExternalInput")
    bias = nc.dram_tensor("bias", [128, BIAS_COLS], FP32, kind="ExternalInput")
    out = nc.dram_tensor("out", [S, D], FP32, kind="ExternalOutput")

    with tile.TileContext(nc) as tc:
        with (
            tc.tile_pool(name="weights", bufs=1) as wpool,
            tc.tile_pool(name="acts", bufs=1) as apool,
            tc.tile_pool(name="exps", bufs=20) as epool,
            tc.tile_pool(name="stages", bufs=6) as spool,
            tc.tile_pool(name="smalls", bufs=6) as smpool,
            tc.tile_pool(name="scores", bufs=2, space="PSUM") as scpool,
            tc.tile_pool(name="attnout", bufs=2, space="PSUM") as aopool,
            tc.tile_pool(name="projacc", bufs=2, space="PSUM") as prpool,
        ):
            # ---- PE warm-up: full-array junk matmuls keep HAM's activity
            # window busy while the first DMAs stream, so the real prefix
            # runs at 2.4GHz. (K=1 slivers don't register as activity; the
            # N=512 streams give ~100% duty cycle. 8 of them = ~3.4us = one
            # full HAM activity window, ending right as the first x chunk
            # lands.)
            wtiny = wpool.tile([128, SC], BF16, name="wtiny")
            nc.vector.memset(wtiny, 0.0)

            def warm_pe(n):
                warm_ps = prpool.tile([128, SC], FP32, name="warm_ps", tag="ps")
                for _ in range(n):
                    nc.tensor.matmul(warm_ps, lhsT=wtiny[:, 0:128], rhs=wtiny)

            warm_pe(14)

            # ---- resident inputs: all on the Sync HWDGE ring, need-ordered.
            xt_all = wpool.tile([128, N_SC * DCH * SC], BF16, name="xt_all")

            def xt(dc, sc):
                base = sc * (DCH * SC) + dc * SC
                return xt_all[:, base:base + SC]

            wk_sb = wpool.tile([128, DCH * FL], BF16, name="wk_sb")
            wkt = [wk_sb[:, dc * FL:(dc + 1) * FL] for dc in range(DCH)]
            wq_sb = wpool.tile([128, DCH * FL], BF16, name="wq_sb")
            wqt = [wq_sb[:, dc * FL:(dc + 1) * FL] for dc in range(DCH)]
            bias_sb = wpool.tile([128, BIAS_COLS], FP32, name="bias_sb")
            bk_sb = bias_sb[:, BK_OFF:BK_OFF + 2]
            bq_sb = bias_sb[:, BQ_OFF:BQ_OFF + 2]
            bv_sb = bias_sb[:, BV_OFF:BV_OFF + FL]
            mb_sb = bias_sb[:, MB_OFF:MB_OFF + N_JC]
            wv_sb = wpool.tile([128, DCH * FL], BF16, name="wv_sb")
            wvt = [wv_sb[:, dc * FL:(dc + 1) * FL] for dc in range(DCH)]
            wo_sb = wpool.tile([128, 2 * D], BF16, name="wo_sb")
            wot = [wo_sb[:, fc * D:(fc + 1) * D] for fc in range(2)]

            half = DCH * SC // 2
            # q before k: the first scores need ALL of q s-chunk 0 but only
            # the first columns of kT, which a column-split k projection
            # produces right as wk (ordered last of the four) lands
            nc.sync.dma_start(out=wq_sb, in_=wq.ap())
            # s-chunk 0 in dc halves so the q projection halves can start as
            # soon as their contraction operands land
            nc.sync.dma_start(out=xt_all[:, 0:half], in_=xP.ap()[:, 0:half])
            nc.sync.dma_start(out=xt_all[:, half:2 * half],
                              in_=xP.ap()[:, half:2 * half])
            nc.sync.dma_start(out=wk_sb, in_=wk.ap())
            nc.sync.dma_start(out=bias_sb, in_=bias.ap())
            nc.sync.dma_start(out=wv_sb, in_=wv.ap())
            for sc in range(1, N_SC):
                nc.sync.dma_start(
                    out=xt_all[:, sc * DCH * SC:(sc + 1) * DCH * SC],
                    in_=xP.ap()[:, sc * DCH * SC:(sc + 1) * DCH * SC],
                )
            nc.sync.dma_start(out=wo_sb, in_=wo.ap())
            # lower half of wot[1] shifted to partitions 0-63: lets the tail
            # out_proj contract head B's staging tile directly (K=64) instead
            # of waiting for the partition-shift DMA into at[1]
            wo_lo = wpool.tile([64, D], BF16, name="wo_lo")
            nc.sync.dma_start(out=wo_lo, in_=wo_sb[64:128, D:2 * D])

            # warm the ScalarE Exp table while DMAs stream
            warm = smpool.tile([1, 4], FP32, name="warm", tag="warm")
            nc.vector.memset(warm, 1.0)
            nc.scalar.activation(warm, warm, mybir.ActivationFunctionType.Exp)

            # ---- persistent activations ----
            qt = [apool.tile([128, S], BF16, name=f"qt{p}") for p in range(2)]
            kt = [apool.tile([128, S], BF16, name=f"kt{p}") for p in range(2)]
            # v natural: tile sc = rows [128sc,128sc+128), layout (128, 4 heads, 65)
            vt = [apool.tile([128, HL, 65], BF16, name=f"vt{sc}") for sc in range(N_JC)]
            at = [apool.tile([128, S], BF16, name=f"at{p}") for p in range(2)]

            qk_open = {}

            def qk_half(dst, w_tiles, bias_ap, sc, fc, half):
                """Half of a qT/kT projection s-chunk (4 of 8 d-accumulation
                matmuls, ~0.9us of PE) so drip slots stay small."""
                key = (id(dst), sc)
                if half == 0:
                    ps = prpool.tile([128, SC], FP32, name="ps", tag="ps")
                    qk_open[key] = ps
                else:
                    ps = qk_open.pop(key)
                for dc in range(half * 4, half * 4 + 4):
                    nc.tensor.matmul(
                        ps,
                        lhsT=w_tiles[dc][:, fc * 128:(fc + 1) * 128],
                        rhs=xt(dc, sc),
                        start=(dc == 0),
                        stop=(dc == DCH - 1),
                    )
                if half == 1:
                    nc.vector.tensor_scalar_add(
                        dst[:, sc * SC:(sc + 1) * SC], ps, bias_ap[:, fc:fc + 1]
                    )

            def qk_full(dst, w_tiles, bias_ap, sc, fc):
                qk_half(dst, w_tiles, bias_ap, sc, fc, 0)
                qk_half(dst, w_tiles, bias_ap, sc, fc, 1)

            def qk_cols(dst, w_tiles, bias_ap, sc, fc, c0, c1):
                """Column slice of a qT/kT projection s-chunk (all 8 dc)."""
                ps = prpool.tile([128, c1 - c0], FP32, name="ps", tag="ps")
                for dc in range(DCH):
                    nc.tensor.matmul(
                        ps,
                        lhsT=w_tiles[dc][:, fc * 128:(fc + 1) * 128],
                        rhs=xt(dc, sc)[:, c0:c1],
                        start=(dc == 0),
                        stop=(dc == DCH - 1),
                    )
                nc.vector.tensor_scalar_add(
                    dst[:, sc * SC + c0:sc * SC + c1], ps, bias_ap[:, fc:fc + 1]
                )

            def v_proj(j):
                """v rows [128j,+128) for all 4 heads (N=256, ~1.05us)."""
                sc, off = j // 4, (j % 4) * JC
                ps = prpool.tile([128, FL], FP32, name="ps", tag="ps")
                for dc in range(DCH):
                    nc.tensor.matmul(
                        ps,
                        lhsT=xt(dc, sc)[:, off:off + JC],
                        rhs=wvt[dc],
                        start=(dc == 0),
                        stop=(dc == DCH - 1),
                    )
                nc.vector.tensor_add(
                    vt[j][:, :, 0:64],
                    ps.rearrange("p (h d) -> p h d", h=HL),
                    bv_sb.rearrange("p (h d) -> p h d", h=HL),
                )
                nc.vector.memset(vt[j][:, :, 64:65], 1.0)

            pending_norm = {}
            last_stg = [None]
            prefetched = [None]

            def emit_scores(pair, i_sl, jc):
                """scoresT = k @ q.T, two heads row-tiled (K=64 each)."""
                sc_ps = scpool.tile([128, 2 * SC], FP32, name="sc_ps")
                nc.tensor.matmul(
                    sc_ps[:, 0:SC],
                    lhsT=kt[pair][0:64, jc * JC:(jc + 1) * JC],
                    rhs=qt[pair][0:64, i_sl],
                )
                nc.tensor.matmul(
                    sc_ps[:, SC:2 * SC],
                    lhsT=kt[pair][64:128, jc * JC:(jc + 1) * JC],
                    rhs=qt[pair][64:128, i_sl],
                )
                return sc_ps

            def attention(pair, per_jc_hook=None, final=False, next_pair=None):
                """Full attention for heads (2*pair, 2*pair+1).

                Scores run one jc ahead of the exp stream, ACROSS block
                boundaries: the next block's jc0 scores are emitted before
                this block's last attn@v, so the first exp of a block never
                waits on the PE catching up at a boundary."""
                for ic in range(N_SC):
                    i_sl = slice(ic * SC, (ic + 1) * SC)
                    outA = aopool.tile([65, SC], FP32, name="outA", tag="ao")
                    outB = aopool.tile([65, SC], FP32, name="outB", tag="ao")
                    sc_cur = prefetched[0]
                    prefetched[0] = None
                    if sc_cur is None:
                        sc_cur = emit_scores(pair, i_sl, 0)
                    for jc in range(N_JC):
                        ex = epool.tile([128, 2 * SC], BF16, name="ex")
                        nc.scalar.activation(
                            ex, sc_cur, mybir.ActivationFunctionType.Exp,
                            bias=mb_sb[:, jc:jc + 1], scale=1.0 / np.sqrt(DH),
                        )
                        # deferred normalize of the PREVIOUS block: the
                        # copy + reciprocal DMA round trip finishes ~5us into
                        # this block, so the GpSimd partition-broadcast +
                        # normalize muls land at jc slots 5-6 with nothing on
                        # the PE stream at all. MUST be emitted before the
                        # hook work (out_proj drips read `at`).
                        for key, slot in (("b", 5), ("c", 6)):
                            if jc == slot and key in pending_norm:
                                pending_norm.pop(key)()
                        if per_jc_hook is not None:
                            per_jc_hook(ic, jc)
                        # next scores AFTER the hook: drips at slot jc may
                        # still be writing the kt/qt columns they read
                        if jc < N_JC - 1:
                            sc_cur = emit_scores(pair, i_sl, jc + 1)
                        elif ic < N_SC - 1:
                            prefetched[0] = emit_scores(
                                pair, slice((ic + 1) * SC, (ic + 2) * SC), 0)
                        elif next_pair is not None:
                            prefetched[0] = emit_scores(next_pair, slice(0, SC), 0)
                        nc.tensor.matmul(
                            outA, lhsT=vt[jc][:, 2 * pair, :], rhs=ex[:, 0:SC],
                            start=(jc == 0), stop=(jc == N_JC - 1),
                        )
                        nc.tensor.matmul(
                            outB, lhsT=vt[jc][:, 2 * pair + 1, :], rhs=ex[:, SC:2 * SC],
                            start=(jc == 0), stop=(jc == N_JC - 1),
                        )
                    # Boundary work kept minimal: bounce the PSUM Z rows to a
                    # (64,8) shape via DMA so the reciprocal runs at 8 elems
                    # per lane (233ns; a 1-partition reciprocal costs 3.3us),
                    # then DMA-cast back to a bf16 row. The ~4us round trip
                    # is hidden: the broadcast matmul + normalize mul are
                    # deferred into the next block's jc slots 3-4. PSUM->SBUF
                    # copies recycle the attn-out banks.
                    osbA = smpool.tile([65, SC], FP32, name="osbA", tag="osb")
                    nc.vector.tensor_copy(osbA, outA)
                    osbB = smpool.tile([65, SC], FP32, name="osbB", tag="osb")
                    nc.vector.tensor_copy(osbB, outB)
                    recA = smpool.tile([1, SC], BF16, name="recA", tag="recbf")
                    recB = smpool.tile([1, SC], BF16, name="recB", tag="recbf")
                    # bounce DMAs ride the Sync HWDGE ring (~0.6us first-byte
                    # vs ~2us on the GpSimd SWDGE path). Both forward bounces
                    # issue before either return bounce: the return waits on
                    # its reciprocal, and the Sync ring is FIFO — issuing
                    # zspB behind recA-back would head-of-line block it.
                    zspA = smpool.tile([64, SC // 64], FP32, name="zspA", tag="zsp")
                    zspB = smpool.tile([64, SC // 64], FP32, name="zspB", tag="zsp")
                    nc.sync.dma_start(out=zspA, in_=osbA[64:65, :])
                    nc.sync.dma_start(out=zspB, in_=osbB[64:65, :])
                    rspA = smpool.tile([64, SC // 64], BF16, name="rspA", tag="rsp")
                    rspB = smpool.tile([64, SC // 64], BF16, name="rspB", tag="rsp")
                    with nc.allow_low_precision(reason="1/Z is bf16 by design"):
                        nc.vector.reciprocal(rspA, zspA)
                        nc.vector.reciprocal(rspB, zspB)
                    nc.sync.dma_start(out=recA, in_=rspA)
                    nc.sync.dma_start(out=recB, in_=rspB)

                    is_final = final and ic == N_SC - 1

                    def t2b(pair=pair, i_sl=i_sl, osbA=osbA, recA=recA):
                        bcs = smpool.tile([64, SC], BF16, name="bcs", tag="bcs")
                        nc.gpsimd.partition_broadcast(bcs, recA)
                        nc.vector.tensor_mul(at[pair][0:64, i_sl], osbA[0:64, :], bcs)

                    def t2c(pair=pair, i_sl=i_sl, osbB=osbB, recB=recB,
                            is_final=is_final):
                        bcs = smpool.tile([64, SC], BF16, name="bcs", tag="bcs")
                        nc.gpsimd.partition_broadcast(bcs, recB)
                        stg = smpool.tile([64, SC], BF16, name="stg", tag="stg")
                        nc.vector.tensor_mul(stg, osbB[0:64, :], bcs)
                        if is_final:
                            # tail out_proj reads stg directly (3-matmul form)
                            last_stg[0] = stg
                        else:
                            # shift to partitions 64..127 (DVE can't cross lanes)
                            nc.sync.dma_start(out=at[pair][64:128, i_sl], in_=stg)

                    pending_norm["b"] = t2b
                    pending_norm["c"] = t2c

            def flush_norm():
                for key in ("b", "c"):
                    if key in pending_norm:
                        pending_norm.pop(key)()

            def out_proj_chunk(ic, ec, ss, dma_eng=None, copy_eng=None):
                """One (128 s, 512 e) chunk of the partial output projection."""
                srow = ic * SC + ss * 128
                po = prpool.tile([128, SC], FP32, name="po", tag="ps")
                for fc in range(2):
                    nc.tensor.matmul(
                        po,
                        lhsT=at[fc][:, srow:srow + 128],
                        rhs=wot[fc][:, ec * SC:(ec + 1) * SC],
                        start=(fc == 0),
                        stop=(fc == 1),
                    )
                stg = spool.tile([128, SC], FP32, name="ostg")
                if copy_eng is nc.scalar:
                    nc.scalar.copy(stg, po)
                else:
                    nc.vector.tensor_copy(stg, po)
                (dma_eng or nc.sync).dma_start(
                    out=out.ap()[srow:srow + 128, ec * SC:(ec + 1) * SC],
                    in_=stg,
                )

            # ---- emission order (drives scheduling priority and the
            # per-engine instruction streams; engines execute in order) ----
            #
            # 8 attention blocks (pair, ic). Projection / out-proj work beyond
            # a minimal prefix drips into the jc loops:
            #   block 0: all v chunks just-in-time (vt[j] by jc=j) + k0
            #            s-chunks just-in-time (sc s by jc=4s) + q0 sc1
            #   blocks 1-4: q0/k1/q1 halves (light; blocks run at exp pace)
            #   blocks 5-7: previous ic's out_proj chunks + q1 leftovers
            K0, Q0, K1, Q1 = (kt[0], wkt, bk_sb, 0), (qt[0], wqt, bq_sb, 0), \
                             (kt[1], wkt, bk_sb, 1), (qt[1], wqt, bq_sb, 1)

            def qk_thunk(args, scn, half):
                dst, w, b, fc = args
                return lambda: qk_half(dst, w, b, scn, fc, half)

            sched = {b: {} for b in range(8)}

            def put(b, jc, thunk):
                sched[b].setdefault(jc, []).append(thunk)

            # block 0: v streaming just-in-time at slot j (attn@v jc waits on
            # ACT(jc) which trails scores(jc) by ~1.1us, so v_proj(j) emitted
            # right after scores(j) still lands in time and never delays the
            # next scores) + k0 halves ahead of their first consumer (kt sc s
            # is read from jc=4s).
            for j in range(0, N_JC):
                put(0, j, lambda j=j: v_proj(j))
            put(0, 2, qk_thunk(K0, 1, 0)); put(0, 3, qk_thunk(K0, 1, 1))
            put(0, 6, qk_thunk(K0, 2, 0)); put(0, 7, qk_thunk(K0, 2, 1))
            put(0, 10, qk_thunk(K0, 3, 0)); put(0, 11, qk_thunk(K0, 3, 1))
            put(0, 13, qk_thunk(Q0, 1, 0)); put(0, 14, qk_thunk(Q0, 1, 1))
            # blocks 1-4: spread remaining projections (all light)
            put(1, 4, qk_thunk(Q0, 2, 0)); put(1, 5, qk_thunk(Q0, 2, 1))
            put(1, 8, qk_thunk(Q0, 3, 0)); put(1, 9, qk_thunk(Q0, 3, 1))
            put(1, 12, qk_thunk(K1, 0, 0)); put(1, 13, qk_thunk(K1, 0, 1))
            put(2, 4, qk_thunk(K1, 1, 0)); put(2, 5, qk_thunk(K1, 1, 1))
            put(2, 8, qk_thunk(K1, 2, 0)); put(2, 9, qk_thunk(K1, 2, 1))
            put(2, 12, qk_thunk(K1, 3, 0)); put(2, 13, qk_thunk(K1, 3, 1))
            put(3, 4, qk_thunk(Q1, 0, 0)); put(3, 5, qk_thunk(Q1, 0, 1))
            put(3, 8, qk_thunk(Q1, 1, 0)); put(3, 9, qk_thunk(Q1, 1, 1))
            put(4, 4, qk_thunk(Q1, 2, 0)); put(4, 5, qk_thunk(Q1, 2, 1))
            put(4, 8, qk_thunk(Q1, 3, 0)); put(4, 9, qk_thunk(Q1, 3, 1))
            # blocks 5-7: drip previous ic's out_proj (8 chunks each; slot 7+
            # so the deferred normalize muls at slots 5-6 land first)
            for b in range(5, 8):
                ic_prev = b - 5
                idx = 0
                for ec in range(2):
                    for ss in range(SC // 128):
                        put(b, 7 + idx, lambda ic=ic_prev, ec=ec, ss=ss:
                            out_proj_chunk(ic, ec, ss))
                        idx += 1

            def hook(block):
                def _h(ic, jc):
                    for thunk in sched[block].get(jc, []):
                        thunk()
                return _h

            # minimal prefix: q0 halves paced by the split sc0 DMA arrival,
            # then k0 in column pieces (the first scores only need kT cols
            # 0:128, so a 256-col piece unblocks them right as wk lands).
            # Junk warm matmuls fill the DMA-wait gaps so the HAM activity
            # window never sees the PE idle and everything runs at 2.4GHz.
            qk_half(qt[0], wqt, bq_sb, 0, 0, 0)
            warm_pe(4)
            qk_half(qt[0], wqt, bq_sb, 0, 0, 1)
            warm_pe(4)
            qk_cols(kt[0], wkt, bk_sb, 0, 0, 0, 256)
            qk_cols(kt[0], wkt, bk_sb, 0, 0, 256, 512)

            attention(0, per_jc_hook=lambda ic, jc: hook(ic)(ic, jc),
                      next_pair=1)
            attention(1, per_jc_hook=lambda ic, jc: hook(4 + ic)(ic, jc),
                      final=True)
            # tail: keep the PE's HAM activity monitor hot through the ~4us
            # reciprocal round trip (else the final out_proj runs at the cold
            # 1.2GHz clock), then the final block's normalize + out_proj.
            # The final out_proj uses the 3-matmul form (head B contracted
            # from the staging tile via wo_lo) so it starts right after the
            # normalize muls instead of waiting for the partition-shift DMA.
            # Copies and DMAs alternate between engines so neither the
            # Vector queue nor one DMA ring serializes the drain.
            warm_pe(34)
            flush_norm()
            idx = 0
            for ec in range(2):
                for ss in range(SC // 128):
                    srow = (N_SC - 1) * SC + ss * 128
                    po = prpool.tile([128, SC], FP32, name="po", tag="ps")
                    e_sl = slice(ec * SC, (ec + 1) * SC)
                    nc.tensor.matmul(po, lhsT=at[0][:, srow:srow + 128],
                                     rhs=wot[0][:, e_sl], start=True, stop=False)
                    nc.tensor.matmul(po, lhsT=at[1][0:64, srow:srow + 128],
                                     rhs=wot[1][0:64, e_sl], start=False, stop=False)
                    nc.tensor.matmul(
                        po, lhsT=last_stg[0][:, ss * 128:(ss + 1) * 128],
                        rhs=wo_lo[:, e_sl], start=False, stop=True)
                    stg = spool.tile([128, SC], FP32, name="ostg")
                    if idx % 2:
                        nc.scalar.copy(stg, po)
                    else:
                        nc.vector.tensor_copy(stg, po)
                    (nc.scalar if idx % 2 else nc.sync).dma_start(
                        out=out.ap()[srow:srow + 128, e_sl], in_=stg)
                    idx += 1

    nc.compile()
    return nc


_NC_CACHE = None


def _get_nc():
    global _NC_CACHE
    if _NC_CACHE is None:
        _NC_CACHE = build_kernel()
    return _NC_CACHE


def make_in_maps(inputs):
    x = np.asarray(inputs["x"], dtype=np.float32)
    mask = np.asarray(inputs["mask"])
    Wq = np.asarray(inputs["Wq"], dtype=np.float32)
    bq = np.asarray(inputs["bq"], dtype=np.float32)
    Wk = np.asarray(inputs["Wk"], dtype=np.float32)
    bk = np.asarray(inputs["bk"], dtype=np.float32)
    Wv = np.asarray(inputs["Wv"], dtype=np.float32)
    bv = np.asarray(inputs["bv"], dtype=np.float32)
    Wo = np.asarray(inputs["Wo"], dtype=np.float32)

    bf = ml_dtypes.bfloat16

    def pack_dxf(wT):  # (1024, FL) -> (128, 8*FL): d-chunks side by side
        return np.ascontiguousarray(
            wT.reshape(DCH, 128, FL).transpose(1, 0, 2).reshape(128, DCH * FL)
        )

    def pack_fxe(woT):  # (256, D) -> (128, 2*D): f-chunks side by side
        return np.ascontiguousarray(
            woT.reshape(2, 128, D).transpose(1, 0, 2).reshape(128, 2 * D)
        )

    in_maps = []
    for c in range(NCORES):
        b = c // GROUPS
        g = c % GROUPS
        fs, fe = g * FL, (g + 1) * FL
        # x s-chunk-major: xP[p, sc*4096 + dc*512 + j] = x[b][sc*512+j, dc*128+p]
        xPc = np.ascontiguousarray(
            x[b].reshape(N_SC, SC, DCH, 128).transpose(3, 0, 2, 1)
            .reshape(128, N_SC * DCH * SC)
        ).astype(bf)
        bias_c = np.zeros((128, BIAS_COLS), dtype=np.float32)
        bias_c[:, BK_OFF:BK_OFF + 2] = bk[fs:fe].reshape(2, 128).T
        bias_c[:, BQ_OFF:BQ_OFF + 2] = bq[fs:fe].reshape(2, 128).T
        bias_c[:, BV_OFF:BV_OFF + FL] = np.tile(bv[fs:fe], (128, 1))
        bias_c[:, MB_OFF:MB_OFF + N_JC] = (
            np.where(mask[b] == 0, np.float32(-1e9), np.float32(0.0))
            .astype(np.float32).reshape(N_JC, 128).T
        )
        in_maps.append({
            "xP": xPc,
            "wk": pack_dxf(Wk[fs:fe, :].T.astype(bf)),
            "wq": pack_dxf(Wq[fs:fe, :].T.astype(bf)),
            "wv": pack_dxf(Wv[fs:fe, :].T.astype(bf)),
            "wo": pack_fxe(Wo[:, fs:fe].T.astype(bf)),
            "bias": np.ascontiguousarray(bias_c),
        })
    return in_maps


def kernel(x, mask, Wq, bq, Wk, bk, Wv, bv, Wo, bo):
    bo = np.asarray(bo, dtype=np.float32)
    nc = _get_nc()
    in_maps = make_in_maps(dict(x=x, mask=mask, Wq=Wq, bq=bq, Wk=Wk, bk=bk,
                                Wv=Wv, bv=bv, Wo=Wo, bo=bo))
    res = run_bass_kernel_spmd(nc, in_maps, core_ids=list(range(NCORES)))
    parts = [np.asarray(r["out"], dtype=np.float32) for r in res.results]
    full = np.empty((B, S, D), dtype=np.float32)
    for b in range(B):
        acc = parts[b * GROUPS].copy()
        for g in range(1, GROUPS):
            acc += parts[b * GROUPS + g]
        full[b] = acc + bo[None, :]
    return full
